# revision 1
# baseline (speedup 1.0000x reference)
"""Trainium2 Bass kernel for AdaptiveGatedAttentionFusion.

Strategy: pure data-parallel over batch B=8 across 8 NeuronCores (no
collectives). Each core processes one sample end-to-end:
  - 6 cross-modal attention modules (Q/K/V 1x1 convs, channel-attention
    scores over spatial dim, softmax, attn@V, output proj)
  - SE blocks, channel l2norm, quality metrics, gating, fusion conv.

All matmuls run in bf16 (weights pre-transposed + pre-cast on host),
accumulation in fp32 PSUM. Residual/softmax/norm arithmetic in fp32.

Key layout trick: Q and K convs are computed with TRANSPOSED outputs
(QT/KT = x^T W^T tiles, [spatial, channels]) so the score Gram matrix
G^T = K Q^T (contraction over spatial) comes out directly with softmax's
normalization axis on PARTITIONS; normalization is done with ones-vector
matmuls on the PE (column sums + row replication), avoiding transposes.
attn^T lands directly in block-diagonal head-pair tiles for attn@V.
"""

import numpy as np
import ml_dtypes

import concourse.bass as bass
import concourse.mybir as mybir
import concourse.tile as tile
from concourse import bacc
from concourse.bass_utils import run_bass_kernel_spmd

BF16 = mybir.dt.bfloat16
F32 = mybir.dt.float32
AF = mybir.ActivationFunctionType
ALU = mybir.AluOpType

B = 8
C = 512
H = 64
W = 64
HW = H * W          # 4096
NB = C // 128       # 4 channel blocks
NT = HW // 512      # 8 n-tiles of 512
NBLK = HW // 128    # 32 n-blocks of 128
HEADS = 8
HD = C // HEADS     # 64
CO = 3 * C          # 1536
NOB = CO // 128     # 12 output-channel blocks for fusion
GIN = 3 * C + 3     # 1539 gate input channels
GCH = 13            # 12 full chunks + 1 quality chunk (padded to 128)
EPS = 1e-6
NTOT = C * HW       # 2097152 elements per modality
SCALE = HD ** -0.5  # 0.125

# module order: (index, query modality, kv modality); grouped per query
# modality so cross accumulators complete in pairs.
MODS = [(0, 0, 1), (1, 0, 2), (2, 1, 0), (3, 1, 2), (4, 2, 0), (5, 2, 1)]

_CACHE = {}
_last_in_maps = None


def _legalize_sync_waits(nc):
    """This container's walrus encodes at most one sync-wait per
    instruction; Tile attaches several. Hoist extra waits onto NoOps
    inserted immediately before the instruction on the same engine."""
    uid = [0]
    for f in nc.m.functions:
        for blk in f.blocks:
            insts = blk.instructions
            if not any(
                getattr(i, "sync_info", None) is not None
                and i.sync_info.on_wait and len(i.sync_info.on_wait) > 1
                for i in insts
            ):
                continue
            out = []
            for inst in insts:
                si = getattr(inst, "sync_info", None)
                if si is not None and si.on_wait and len(si.on_wait) > 1:
                    waits = list(si.on_wait)
                    for w in waits[:-1]:
                        uid[0] += 1
                        nop = mybir.InstNoOp(
                            name=f"I-syncw-{uid[0]}", ins=[], outs=[]
                        )
                        nop.engine = inst.engine
                        nop.sync_info = mybir.SyncInfo(
                            on_wait=[w], on_update=[]
                        )
                        out.append(nop)
                    si.on_wait = [waits[-1]]
                out.append(inst)
            insts[:] = out


def _build(flags, nmods=6, do_final=True, fin_stage=4, q_stage=4):
    (bq_nz, bk_nz, bv_nz, crossb_nz, seb1_nz, seb2_nz, gb_nz, fb_nz) = flags
    nc = bacc.Bacc()

    x_d = [nc.declare_dram_parameter(f"x{m}", [C, HW], BF16, isOutput=False) for m in range(3)]
    wqt_d = nc.declare_dram_parameter("wqt", [6, C, C], BF16, isOutput=False)
    wkt_d = nc.declare_dram_parameter("wkt", [6, C, C], BF16, isOutput=False)
    wvt_d = nc.declare_dram_parameter("wvt", [6, C, C], BF16, isOutput=False)
    wot_d = nc.declare_dram_parameter("wot", [6, C, C], BF16, isOutput=False)
    bq_d = nc.declare_dram_parameter("bq", [6, C], F32, isOutput=False)
    bk_d = nc.declare_dram_parameter("bk", [6, C], F32, isOutput=False)
    bv_d = nc.declare_dram_parameter("bv", [6, C], F32, isOutput=False)
    crossb_d = nc.declare_dram_parameter("crossb", [3, C], F32, isOutput=False)
    sew1t_d = nc.declare_dram_parameter("sew1t", [3, C, 32], BF16, isOutput=False)
    sew2t_d = nc.declare_dram_parameter("sew2t", [3, 32, C], BF16, isOutput=False)
    seb1_d = nc.declare_dram_parameter("seb1", [3, 32], F32, isOutput=False)
    seb2_d = nc.declare_dram_parameter("seb2", [3, C], F32, isOutput=False)
    gwt_d = nc.declare_dram_parameter("gwt", [GCH * 128, 96], BF16, isOutput=False)
    gb_d = nc.declare_dram_parameter("gb", [96], F32, isOutput=False)
    # fusion weight pre-packed as [o_blk, c_blk, 128c, 128o]
    fwt_d = nc.declare_dram_parameter("fwt", [NOB, NOB, 128, 128], BF16, isOutput=False)
    fb_d = nc.declare_dram_parameter("fb", [CO], F32, isOutput=False)
    out_d = nc.declare_dram_parameter("out", [CO, HW], F32, isOutput=True)

    cross_d = [nc.dram_tensor(f"cross{m}", [C, HW], BF16) for m in range(3)]

    with tile.TileContext(nc) as tc:
        with (
            tc.tile_pool(name="pbig", bufs=1) as pbig,
            tc.tile_pool(name="pcr", bufs=1) as pcr,
            tc.tile_pool(name="pqk", bufs=3) as pqk,
            tc.tile_pool(name="pw", bufs=1) as pw,
            tc.tile_pool(name="patt", bufs=2) as patt,
            tc.tile_pool(name="pstream", bufs=4) as pstream,
            tc.tile_pool(name="pfw", bufs=16) as pfw,
            tc.tile_pool(name="pmisc", bufs=1) as pmisc,
            tc.tile_pool(name="pfin", bufs=3) as pfin,
            tc.tile_pool(name="psum", bufs=8, space="PSUM") as psum,
        ):
            # ---------- constants ----------
            ones_col = pmisc.tile([128, 1], BF16, tag="ones_col")
            nc.vector.memset(ones_col, 1.0)
            ones_col_f = pmisc.tile([128, 1], F32, tag="ones_col_f")
            nc.vector.memset(ones_col_f, 1.0)
            ones_k1_f = pmisc.tile([1, 128], F32, tag="ones_k1_f")
            nc.vector.memset(ones_k1_f, 1.0)
            ones_f32 = pmisc.tile([4, 512], F32, tag="ones_f32")
            nc.vector.memset(ones_f32, 1.0)

            # ---------- load x residents ----------
            xt = []  # xt[m][c] : [128, HW] bf16
            for m in range(3):
                tiles = []
                for c in range(NB):
                    t = pbig.tile([128, HW], BF16, tag=f"big{m * NB + c}")
                    nc.sync.dma_start(out=t, in_=x_d[m][c * 128:(c + 1) * 128, :])
                    tiles.append(t)
                xt.append(tiles)

            # optional row-bias tiles for QT/KT drains (bias varies along the
            # free dim in transposed layout -> need broadcast rows)
            def row_bias(dram_row, tag):
                bt = pmisc.tile([128, C], F32, tag=tag)
                src = bass.AP(
                    tensor=dram_row.tensor,
                    offset=dram_row.offset,
                    ap=[[0, 128]] + dram_row.ap,
                )
                nc.gpsimd.dma_start(out=bt, in_=src)
                return bt

            # ---------- per-module pipeline ----------
            cross = {}  # modality -> list of 4 [128,HW] bf16 tiles
            for mi, (i, mq, mk) in enumerate(MODS[:nmods]):
                first_of_pair = (mi % 2) == 0
                if first_of_pair:
                    # init cross accumulator for this query modality
                    crt = []
                    for c in range(NB):
                        t = pcr.tile([128, HW], BF16, tag=f"cr{c}")
                        if crossb_nz:
                            cb = pmisc.tile([128, 1], F32, tag="crossb_t")
                            nc.sync.dma_start(
                                out=cb,
                                in_=crossb_d[mq, c * 128:(c + 1) * 128].rearrange(
                                    "a -> a 1"
                                ),
                            )
                            nc.vector.tensor_scalar_add(t, xt[mq][c], cb)
                        else:
                            nc.sync.dma_start(out=t, in_=xt[mq][c])
                        crt.append(t)
                    cross[mq] = crt

                # weights for this module
                wq = []
                wk = []
                wv = []
                wo = []
                for c in range(NB):
                    for lst, src, nm in (
                        (wq, wqt_d, "wq"), (wk, wkt_d, "wk"),
                        (wv, wvt_d, "wv"), (wo, wot_d, "wo"),
                    ):
                        t = pw.tile([128, C], BF16, tag=f"{nm}{c}")
                        nc.sync.dma_start(
                            out=t, in_=src[i, c * 128:(c + 1) * 128, :]
                        )
                        lst.append(t)

                bq_t = row_bias(bq_d[i, :], "bq_row") if bq_nz else None
                bk_t = row_bias(bk_d[i, :], "bk_row") if bk_nz else None

                # ---- phase A: QT/KT convs + Gram^T accumulation ----
                gram = [psum.tile([128, 512], F32, tag="mm", name=f"gram{g}")
                        for g in range(NB)]
                for nb in range(NBLK):
                    ns = slice(nb * 128, (nb + 1) * 128)
                    psq = psum.tile([128, 512], F32, tag="mm")
                    for c in range(NB):
                        nc.tensor.matmul(
                            psq, lhsT=xt[mq][c][:, ns], rhs=wq[c],
                            start=(c == 0), stop=(c == NB - 1),
                        )
                    qt_sb = pqk.tile([128, 512], BF16, tag="qt")
                    if bq_nz:
                        nc.vector.tensor_add(qt_sb, psq, bq_t)
                    else:
                        nc.scalar.copy(out=qt_sb, in_=psq)

                    psk = psum.tile([128, 512], F32, tag="mm")
                    for c in range(NB):
                        nc.tensor.matmul(
                            psk, lhsT=xt[mk][c][:, ns], rhs=wk[c],
                            start=(c == 0), stop=(c == NB - 1),
                        )
                    kt_sb = pqk.tile([128, 512], BF16, tag="kt")
                    if bk_nz:
                        nc.vector.tensor_add(kt_sb, psk, bk_t)
                    else:
                        nc.vector.tensor_copy(out=kt_sb, in_=psk)

                    # Gram^T[a,b] += sum_n KT[n,a] QT[n,b]
                    for mc in range(NB):
                        nc.tensor.matmul(
                            gram[mc],
                            lhsT=kt_sb[:, mc * 128:(mc + 1) * 128],
                            rhs=qt_sb,
                            start=(nb == 0), stop=(nb == NBLK - 1),
                        )

                # ---- phase B: softmax -> attn^T block-diagonal pair tiles ----
                pair = []
                for p in range(NB):
                    pt = patt.tile([128, 128], BF16, tag=f"pd{p}")
                    nc.vector.memset(pt, 0.0)
                    pair.append(pt)
                for h in range(HEADS):
                    mc = h // 2
                    r0 = (h % 2) * HD
                    blk = gram[mc][r0:r0 + HD, h * HD:(h + 1) * HD]
                    e_sb = pstream.tile([HD, HD], F32, tag="esb")
                    nc.scalar.activation(
                        out=e_sb, in_=blk, func=AF.Exp, scale=SCALE
                    )
                    pss = psum.tile([1, HD], F32, tag="mm")
                    nc.tensor.matmul(pss, lhsT=ones_col_f[:HD, :], rhs=e_sb,
                                     start=True, stop=True)
                    r_sb = pstream.tile([1, HD], F32, tag="rsb")
                    nc.vector.reciprocal(out=r_sb, in_=pss)
                    psr = psum.tile([HD, HD], F32, tag="mm")
                    nc.tensor.matmul(psr, lhsT=ones_k1_f[:, :HD], rhs=r_sb,
                                     start=True, stop=True)
                    nc.vector.tensor_mul(
                        pair[mc][r0:r0 + HD, r0:r0 + HD], e_sb, psr
                    )

                bv_t = None
                if bv_nz:
                    bv_t = pmisc.tile([128, NB], F32, tag="bv_t")
                    nc.sync.dma_start(
                        out=bv_t,
                        in_=bv_d[i, :].rearrange("(c p) -> p c", p=128),
                    )

                # ---- phase C: V conv + attn@V + out-proj, per n-tile ----
                for nt in range(NT):
                    ts_ = slice(nt * 512, (nt + 1) * 512)
                    ao = []
                    for p in range(NB):
                        psv = psum.tile([128, 512], F32, tag="mm")
                        for c in range(NB):
                            nc.tensor.matmul(
                                psv,
                                lhsT=wv[c][:, p * 128:(p + 1) * 128],
                                rhs=xt[mk][c][:, ts_],
                                start=(c == 0), stop=(c == NB - 1),
                            )
                        v_sb = pstream.tile([128, 512], BF16, tag="vsb", bufs=3)
                        if bv_nz:
                            nc.vector.tensor_scalar_add(
                                v_sb, psv, bv_t[:, p:p + 1]
                            )
                        else:
                            nc.scalar.copy(out=v_sb, in_=psv)
                        psa = psum.tile([128, 512], F32, tag="mm")
                        nc.tensor.matmul(psa, lhsT=pair[p], rhs=v_sb,
                                         start=True, stop=True)
                        ao_sb = pstream.tile([128, 512], BF16, tag=f"ao{p}", bufs=2)
                        nc.vector.tensor_copy(out=ao_sb, in_=psa)
                        ao.append(ao_sb)
                    for o in range(NB):
                        psp = psum.tile([128, 512], F32, tag="mm")
                        for p in range(NB):
                            nc.tensor.matmul(
                                psp,
                                lhsT=wo[p][:, o * 128:(o + 1) * 128],
                                rhs=ao[p],
                                start=(p == 0), stop=(p == NB - 1),
                            )
                        nc.vector.tensor_add(
                            cross[mq][o][:, ts_], psp, cross[mq][o][:, ts_]
                        )

                if mi % 2 == 1:
                    # cross accumulator complete -> spill to DRAM scratch
                    for c in range(NB):
                        nc.sync.dma_start(
                            out=cross_d[mq][c * 128:(c + 1) * 128, :],
                            in_=cross[mq][c],
                        )

            if not do_final:
                for c in range(NB):
                    nc.gpsimd.dma_start(
                        out=out_d[c * 128:(c + 1) * 128, :],
                        in_=cross[0][c])
            else:
                if fin_stage >= 1:
                    # ---------- quality metrics (from bf16 residents) ----------
                    # per-modality scalars: depth zeros count, rgb var, lidar zeros
                    scr = pmisc.tile([128, 512], BF16, tag="q_scr")
                    red = pmisc.tile([128, 3 * NB], F32, tag="q_red")  # per-cblk sums
                    red2 = pmisc.tile([128, NB], F32, tag="q_red2")    # rgb sum x
                    zacc = pmisc.tile([128, NT], F32, tag="q_zacc")
                    sq_part = pmisc.tile([128, 512], BF16, tag="q_sqp")
                    if q_stage >= 1:
                        for slot, xm in ((0, 1), (2, 2)):  # depth zeros, lidar zeros
                            for c in range(NB):
                                for j in range(NT):
                                    ts_ = slice(j * 512, (j + 1) * 512)
                                    nc.vector.tensor_scalar(
                                        scr, xt[xm][c][:, ts_], 0.0, None,
                                        op0=ALU.is_equal,
                                    )
                                    nc.vector.reduce_sum(
                                        out=zacc[:, j:j + 1], in_=scr,
                                        axis=mybir.AxisListType.X,
                                    )
                                nc.vector.reduce_sum(
                                    out=red[:, slot * NB + c:slot * NB + c + 1],
                                    in_=zacc, axis=mybir.AxisListType.X,
                                )
                    if q_stage >= 2:
                        for c in range(NB):  # rgb sum-of-squares and sum
                            acc = pmisc.tile([128, NT], F32, tag="q_chain")
                            for j in range(NT):
                                ts_ = slice(j * 512, (j + 1) * 512)
                                nc.vector.tensor_mul(
                                    sq_part, xt[0][c][:, ts_], xt[0][c][:, ts_]
                                )
                                nc.vector.reduce_sum(
                                    out=acc[:, j:j + 1], in_=sq_part,
                                    axis=mybir.AxisListType.X,
                                )
                            nc.vector.reduce_sum(
                                out=red[:, NB + c:NB + c + 1], in_=acc,
                                axis=mybir.AxisListType.X,
                            )
                            nc.vector.reduce_sum(
                                out=red2[:, c:c + 1], in_=xt[0][c],
                                axis=mybir.AxisListType.X,
                            )
                    if q_stage >= 3:
                        # column-sum the [128, *] partial sums via ones matmul
                        psq1 = psum.tile([1, 3 * NB], F32, tag="mm")
                        nc.tensor.matmul(psq1, lhsT=ones_col_f, rhs=red,
                                         start=True, stop=True)
                        psq2 = psum.tile([1, NB], F32, tag="mm")
                        nc.tensor.matmul(psq2, lhsT=ones_col_f, rhs=red2,
                                         start=True, stop=True)
                        qsc = pmisc.tile([1, 8], F32, tag="q_sc")
                        # [0]=dep_zeros, [1]=rgb_sumsq, [2]=lid_zeros, [3]=rgb_sum
                        nc.vector.reduce_sum(out=qsc[:, 0:1], in_=psq1[:, 0:NB],
                                             axis=mybir.AxisListType.X)
                        nc.vector.reduce_sum(out=qsc[:, 1:2], in_=psq1[:, NB:2 * NB],
                                             axis=mybir.AxisListType.X)
                        nc.vector.reduce_sum(out=qsc[:, 2:3], in_=psq1[:, 2 * NB:3 * NB],
                                             axis=mybir.AxisListType.X)
                        nc.vector.reduce_sum(out=qsc[:, 3:4], in_=psq2,
                                             axis=mybir.AxisListType.X)
                        # var = (S2 - S1^2/N) / (N-1)
                        nc.vector.tensor_mul(qsc[:, 4:5], qsc[:, 3:4], qsc[:, 3:4])
                        nc.vector.tensor_scalar(
                            qsc[:, 4:5], qsc[:, 4:5], -1.0 / NTOT, None, op0=ALU.mult
                        )
                        nc.vector.tensor_add(qsc[:, 4:5], qsc[:, 4:5], qsc[:, 1:2])
                        nc.vector.tensor_scalar(
                            qsc[:, 4:5], qsc[:, 4:5], 1.0 / (NTOT - 1), None, op0=ALU.mult
                        )
                        # quality chunk [128,512]: row0 dep_sparsity, row1 rgb_var,
                        # row2 lid_sparsity, rows 3-127 zero; constant along n.
                    if q_stage >= 4:
                        qual = pmisc.tile([128, 512], BF16, tag="qual")
                        nc.vector.memset(qual, 0.0)
                        nc.vector.tensor_scalar(
                            qual[0:1, :], ones_f32[0:1, :], qsc[:, 0:1], 1.0 / NTOT,
                            op0=ALU.mult, op1=ALU.mult,
                        )
                        nc.vector.tensor_scalar(
                            qual[32:33, :], ones_f32[0:1, :], qsc[:, 4:5], None,
                            op0=ALU.mult,
                        )
                        nc.vector.tensor_scalar(
                            qual[64:65, :], ones_f32[0:1, :], qsc[:, 2:3], 1.0 / NTOT,
                            op0=ALU.mult, op1=ALU.mult,
                        )

                if fin_stage >= 2:
                    # ---------- final phase: SE, l2norm, gate, fusion ----------
                    refined = []  # [m][c] -> [128, HW] bf16 (reuses x resident slots)
                    for m in range(3):
                        tiles = []
                        for c in range(NB):
                            t = pbig.tile([128, HW], BF16, tag=f"big{m * NB + c}")
                            nc.sync.dma_start(
                                out=t, in_=cross_d[m][c * 128:(c + 1) * 128, :]
                            )
                            tiles.append(t)
                        refined.append(tiles)

                    for m in range(3):
                        # SE block
                        pooled = pmisc.tile([128, NB], F32, tag="se_pool")
                        for c in range(NB):
                            nc.vector.reduce_sum(
                                out=pooled[:, c:c + 1], in_=refined[m][c],
                                axis=mybir.AxisListType.X,
                            )
                        pooled_bf = pmisc.tile([128, NB], BF16, tag="se_pool_bf")
                        nc.vector.tensor_copy(out=pooled_bf, in_=pooled)
                        w1 = pmisc.tile([128, NB, 32], BF16, tag="se_w1")
                        nc.sync.dma_start(
                            out=w1,
                            in_=sew1t_d[m].rearrange("(c p) o -> p c o", p=128),
                        )
                        ps_y = psum.tile([32, 1], F32, tag="mm")
                        for c in range(NB):
                            nc.tensor.matmul(
                                ps_y, lhsT=w1[:, c, :],
                                rhs=pooled_bf[:, c:c + 1],
                                start=(c == 0), stop=(c == NB - 1),
                            )
                        y_sb = pmisc.tile([32, 1], BF16, tag="se_y")
                        if seb1_nz:
                            b1t = pmisc.tile([32, 1], F32, tag="se_b1")
                            nc.sync.dma_start(
                                out=b1t, in_=seb1_d[m, :].rearrange("a -> a 1")
                            )
                            nc.scalar.activation(out=y_sb, in_=ps_y, func=AF.Relu,
                                                 scale=1.0 / HW, bias=b1t)
                        else:
                            nc.scalar.activation(out=y_sb, in_=ps_y, func=AF.Relu,
                                                 scale=1.0 / HW)
                        w2 = pmisc.tile([32, C], BF16, tag="se_w2")
                        nc.sync.dma_start(out=w2, in_=sew2t_d[m])
                        scale_sb = pmisc.tile([128, NB], F32, tag="se_scale")
                        for c in range(NB):
                            ps_s = psum.tile([128, 1], F32, tag="mm")
                            nc.tensor.matmul(
                                ps_s, lhsT=w2[:, c * 128:(c + 1) * 128], rhs=y_sb,
                                start=True, stop=True,
                            )
                            if seb2_nz:
                                b2t = pmisc.tile([128, NB], F32, tag="se_b2")
                                nc.sync.dma_start(
                                    out=b2t[:, c:c + 1],
                                    in_=seb2_d[m, c * 128:(c + 1) * 128].rearrange(
                                        "a -> a 1"
                                    ),
                                )
                                nc.scalar.activation(out=scale_sb[:, c:c + 1],
                                                     in_=ps_s, func=AF.Sigmoid,
                                                     scale=1.0, bias=b2t[:, c:c + 1])
                            else:
                                nc.scalar.activation(out=scale_sb[:, c:c + 1],
                                                     in_=ps_s, func=AF.Sigmoid)
                        # refined_pre = cross * se_scale (in place, bf16)
                        for c in range(NB):
                            nc.vector.tensor_scalar_mul(
                                refined[m][c], refined[m][c], scale_sb[:, c:c + 1]
                            )
                        # l2 norm over channels, processed per n-tile
                        for nt in range(NT):
                            ts_ = slice(nt * 512, (nt + 1) * 512)
                            ps_ss = psum.tile([1, 512], F32, tag="mm")
                            for c in range(NB):
                                sq = pqk.tile([128, 512], BF16, tag="qt")
                                nc.vector.tensor_mul(
                                    sq, refined[m][c][:, ts_], refined[m][c][:, ts_]
                                )
                                nc.tensor.matmul(ps_ss, lhsT=ones_col, rhs=sq,
                                                 start=(c == 0), stop=(c == NB - 1))
                            ssn = pstream.tile([1, 512], F32, tag="ssn", bufs=2)
                            nc.scalar.activation(out=ssn, in_=ps_ss, func=AF.Sqrt)
                            nc.vector.tensor_scalar_add(ssn, ssn, EPS)
                            nc.vector.reciprocal(out=ssn, in_=ssn)
                            ps_rep = psum.tile([128, 512], F32, tag="mm")
                            nc.tensor.matmul(ps_rep, lhsT=ones_k1_f, rhs=ssn,
                                             start=True, stop=True)
                            for c in range(NB):
                                nc.vector.tensor_mul(
                                    refined[m][c][:, ts_], refined[m][c][:, ts_],
                                    ps_rep,
                                )

                if fin_stage >= 3:
                    # gate conv -> alpha [3, HW]
                    gw = []
                    for ch in range(GCH):
                        t = pmisc.tile([128, 96], BF16, tag=f"gw{ch}")
                        nc.sync.dma_start(out=t, in_=gwt_d[ch * 128:(ch + 1) * 128, :])
                        gw.append(t)
                    gb_t = None
                    if gb_nz:
                        gb_t = pmisc.tile([96, 1], F32, tag="gb_t")
                        nc.sync.dma_start(out=gb_t, in_=gb_d.rearrange("a -> a 1"))
                    for nt in range(NT):
                        ts_ = slice(nt * 512, (nt + 1) * 512)
                        ps_g = psum.tile([96, 512], F32, tag="mm")
                        for ch in range(GCH):
                            if ch < 12:
                                rhs = refined[ch // NB][ch % NB][:, ts_]
                            else:
                                rhs = qual
                            nc.tensor.matmul(ps_g, lhsT=gw[ch], rhs=rhs,
                                             start=(ch == 0), stop=(ch == GCH - 1))
                        for m in range(3):
                            a_nt = pstream.tile([1, 512], F32, tag=f"a{m}",
                                                name=f"a{m}", bufs=2)
                            if gb_nz:
                                nc.scalar.activation(out=a_nt, in_=ps_g[32 * m:32 * m + 1, :],
                                                     func=AF.Sigmoid, scale=1.0,
                                                     bias=gb_t[32 * m:32 * m + 1, :])
                            else:
                                nc.scalar.activation(out=a_nt, in_=ps_g[32 * m:32 * m + 1, :],
                                                     func=AF.Sigmoid)
                            ps_ar = psum.tile([128, 512], F32, tag="mm")
                            nc.tensor.matmul(ps_ar, lhsT=ones_k1_f, rhs=a_nt,
                                             start=True, stop=True)
                            for c in range(NB):
                                nc.vector.tensor_mul(
                                    refined[m][c][:, ts_], refined[m][c][:, ts_],
                                    ps_ar,
                                )

                if fin_stage >= 4:
                    # fusion conv: out = fusion_W @ weighted (+ fb)
                    fb_t = None
                    if fb_nz:
                        fb_t = pmisc.tile([128, NOB], F32, tag="fb_t")
                        nc.sync.dma_start(
                            out=fb_t, in_=fb_d.rearrange("(o p) -> p o", p=128)
                        )
                    for o in range(NOB):
                        fw = []
                        for cb in range(NOB):
                            t = pfw.tile([128, 128], BF16, tag="fw")
                            nc.sync.dma_start(out=t, in_=fwt_d[o, cb])
                            fw.append(t)
                        for nt in range(NT):
                            ts_ = slice(nt * 512, (nt + 1) * 512)
                            ps_f = psum.tile([128, 512], F32, tag="mm")
                            for cb in range(NOB):
                                nc.tensor.matmul(
                                    ps_f, lhsT=fw[cb],
                                    rhs=refined[cb // NB][cb % NB][:, ts_],
                                    start=(cb == 0), stop=(cb == NOB - 1),
                                )
                            o_sb = pfin.tile([128, 512], F32, tag="osb")
                            if fb_nz:
                                nc.vector.tensor_scalar_add(o_sb, ps_f,
                                                            fb_t[:, o:o + 1])
                            elif (o + nt) % 2 == 0:
                                nc.vector.tensor_copy(out=o_sb, in_=ps_f)
                            else:
                                nc.scalar.copy(out=o_sb, in_=ps_f)
                            nc.sync.dma_start(
                                out=out_d[o * 128:(o + 1) * 128, ts_], in_=o_sb
                            )
    nc.finalize()
    return nc


def _get_graph(flags):
    if flags not in _CACHE:
        _CACHE[flags] = _build(flags)
    return _CACHE[flags]


def kernel(**inputs):
    bf16 = ml_dtypes.bfloat16
    f32 = np.float32

    rgb = np.asarray(inputs["rgb_features"], f32)
    dep = np.asarray(inputs["depth_features"], f32)
    lid = np.asarray(inputs["lidar_features"], f32)
    Wq = np.asarray(inputs["attn_Wq"], f32)
    bq = np.asarray(inputs["attn_bq"], f32)
    Wk = np.asarray(inputs["attn_Wk"], f32)
    bk = np.asarray(inputs["attn_bk"], f32)
    Wv = np.asarray(inputs["attn_Wv"], f32)
    bv = np.asarray(inputs["attn_bv"], f32)
    Wo = np.asarray(inputs["attn_Wo"], f32)
    bo = np.asarray(inputs["attn_bo"], f32)
    seW1 = np.asarray(inputs["se_W1"], f32)
    seb1 = np.asarray(inputs["se_b1"], f32)
    seW2 = np.asarray(inputs["se_W2"], f32)
    seb2 = np.asarray(inputs["se_b2"], f32)
    gW = np.asarray(inputs["gate_W"], f32)
    gb = np.asarray(inputs["gate_b"], f32)
    fW = np.asarray(inputs["fusion_W"], f32)
    fb = np.asarray(inputs["fusion_b"], f32)

    gb96 = np.zeros(96, f32)
    gb96[[0, 32, 64]] = gb
    crossb = np.stack([bo[0] + bo[1], bo[2] + bo[3], bo[4] + bo[5]])
    flags = (
        bool(bq.any()), bool(bk.any()), bool(bv.any()), bool(crossb.any()),
        bool(seb1.any()), bool(seb2.any()), bool(gb.any()), bool(fb.any()),
    )
    nc = _get_graph(flags)

    wqt = np.ascontiguousarray(Wq.transpose(0, 2, 1)).astype(bf16)
    wkt = np.ascontiguousarray(Wk.transpose(0, 2, 1)).astype(bf16)
    wvt = np.ascontiguousarray(Wv.transpose(0, 2, 1)).astype(bf16)
    wot = np.ascontiguousarray(Wo.transpose(0, 2, 1)).astype(bf16)
    sew1t = np.ascontiguousarray(seW1.transpose(0, 2, 1)).astype(bf16)
    sew2t = np.ascontiguousarray(seW2.transpose(0, 2, 1)).astype(bf16)
    gwt = np.zeros((GCH * 128, 96), f32)
    for m3 in range(3):
        gwt[:3 * C, 32 * m3] = gW.T[:3 * C, m3]
        for q3 in range(3):
            gwt[12 * 128 + 32 * q3, 32 * m3] = gW.T[3 * C + q3, m3]
    gwt = gwt.astype(bf16)
    fwt = np.ascontiguousarray(
        fW.T.reshape(NOB, 128, NOB, 128).transpose(2, 0, 1, 3)
    ).astype(bf16)

    shared = {
        "wqt": wqt, "wkt": wkt, "wvt": wvt, "wot": wot,
        "bq": bq, "bk": bk, "bv": bv, "crossb": crossb,
        "sew1t": sew1t, "sew2t": sew2t, "seb1": seb1, "seb2": seb2,
        "gwt": gwt, "gb": gb96, "fwt": fwt, "fb": fb,
    }
    in_maps = []
    for b in range(B):
        m = dict(shared)
        m["x0"] = rgb[b].reshape(C, HW).astype(bf16)
        m["x1"] = dep[b].reshape(C, HW).astype(bf16)
        m["x2"] = lid[b].reshape(C, HW).astype(bf16)
        in_maps.append(m)

    global _last_in_maps
    _last_in_maps = in_maps
    res = run_bass_kernel_spmd(nc, in_maps, core_ids=list(range(B)))
    out = np.stack([res.results[b]["out"] for b in range(B)])
    return out.reshape(B, CO, H, W).astype(np.float32)



# revision 12
# speedup vs baseline: 1.1521x; 1.1521x over previous
"""Trainium2 Bass kernel for AdaptiveGatedAttentionFusion.

Strategy: pure data-parallel over batch B=8 across 8 NeuronCores (no
collectives). Each core processes one sample end-to-end:
  - 6 cross-modal attention modules (Q/K/V 1x1 convs, channel-attention
    scores over spatial dim, softmax, attn@V, output proj)
  - SE blocks, channel l2norm, quality metrics, gating, fusion conv.

All matmuls run in bf16 (weights pre-transposed + pre-cast on host),
accumulation in fp32 PSUM. Residual/softmax/norm arithmetic in fp32.

Key layout trick: Q and K convs are computed with TRANSPOSED outputs
(QT/KT = x^T W^T tiles, [spatial, channels]) so the score Gram matrix
G^T = K Q^T (contraction over spatial) comes out directly with softmax's
normalization axis on PARTITIONS; normalization is done with ones-vector
matmuls on the PE (column sums + row replication), avoiding transposes.
attn^T lands directly in block-diagonal head-pair tiles for attn@V.
"""

import numpy as np
import ml_dtypes

import concourse.bass as bass
import concourse.mybir as mybir
import concourse.tile as tile
from concourse import bacc
from concourse.bass_utils import run_bass_kernel_spmd

BF16 = mybir.dt.bfloat16
F32 = mybir.dt.float32
AF = mybir.ActivationFunctionType
ALU = mybir.AluOpType

B = 8
C = 512
H = 64
W = 64
HW = H * W          # 4096
NB = C // 128       # 4 channel blocks
NT = HW // 512      # 8 n-tiles of 512
NBLK = HW // 128    # 32 n-blocks of 128
HEADS = 8
HD = C // HEADS     # 64
CO = 3 * C          # 1536
NOB = CO // 128     # 12 output-channel blocks for fusion
GIN = 3 * C + 3     # 1539 gate input channels
GCH = 13            # 12 full chunks + 1 quality chunk (padded to 128)
EPS = 1e-6
NTOT = C * HW       # 2097152 elements per modality
SCALE = HD ** -0.5  # 0.125

# module order: (index, query modality, kv modality); grouped per query
# modality so cross accumulators complete in pairs.
MODS = [(0, 0, 1), (1, 0, 2), (2, 1, 0), (3, 1, 2), (4, 2, 0), (5, 2, 1)]

_CACHE = {}
_last_in_maps = None


def _legalize_sync_waits(nc):
    """This container's walrus encodes at most one sync-wait per
    instruction; Tile attaches several. Hoist extra waits onto NoOps
    inserted immediately before the instruction on the same engine."""
    uid = [0]
    for f in nc.m.functions:
        for blk in f.blocks:
            insts = blk.instructions
            if not any(
                getattr(i, "sync_info", None) is not None
                and i.sync_info.on_wait and len(i.sync_info.on_wait) > 1
                for i in insts
            ):
                continue
            out = []
            for inst in insts:
                si = getattr(inst, "sync_info", None)
                if si is not None and si.on_wait and len(si.on_wait) > 1:
                    waits = list(si.on_wait)
                    for w in waits[:-1]:
                        uid[0] += 1
                        nop = mybir.InstNoOp(
                            name=f"I-syncw-{uid[0]}", ins=[], outs=[]
                        )
                        nop.engine = inst.engine
                        nop.sync_info = mybir.SyncInfo(
                            on_wait=[w], on_update=[]
                        )
                        out.append(nop)
                    si.on_wait = [waits[-1]]
                out.append(inst)
            insts[:] = out


def _build(flags, nmods=6, do_final=True, fin_stage=4, q_stage=4):
    (bq_nz, bk_nz, bv_nz, crossb_nz, seb1_nz, seb2_nz, gb_nz, fb_nz) = flags
    # With zero q/k biases, scores^T = Wk (X_k X_q^T) Wq^T: the pairwise
    # channel grams S are shared across modules, replacing the per-module
    # Q/K convs + spatial gram (6x 3.2 GMAC) with 3 shared S passes plus
    # tiny per-module projections.
    fast_qk = not (bq_nz or bk_nz)
    nc = bacc.Bacc()

    x_d = [nc.declare_dram_parameter(f"x{m}", [C, HW], BF16, isOutput=False) for m in range(3)]
    xT_d = [nc.declare_dram_parameter(f"xt{m}", [HW, C], BF16, isOutput=False) for m in range(3)]
    wqt_d = nc.declare_dram_parameter("wqt", [6, C, C], BF16, isOutput=False)
    wkt_d = nc.declare_dram_parameter("wkt", [6, C, C], BF16, isOutput=False)
    wvt_d = nc.declare_dram_parameter("wvt", [6, C, C], BF16, isOutput=False)
    wot_d = nc.declare_dram_parameter("wot", [6, C, C], BF16, isOutput=False)
    bq_d = nc.declare_dram_parameter("bq", [6, C], F32, isOutput=False)
    bk_d = nc.declare_dram_parameter("bk", [6, C], F32, isOutput=False)
    bv_d = nc.declare_dram_parameter("bv", [6, C], F32, isOutput=False)
    crossb_d = nc.declare_dram_parameter("crossb", [3, C], F32, isOutput=False)
    sew1t_d = nc.declare_dram_parameter("sew1t", [3, C, 32], BF16, isOutput=False)
    sew2t_d = nc.declare_dram_parameter("sew2t", [3, 32, C], BF16, isOutput=False)
    seb1_d = nc.declare_dram_parameter("seb1", [3, 32], F32, isOutput=False)
    seb2_d = nc.declare_dram_parameter("seb2", [3, C], F32, isOutput=False)
    gwt_d = nc.declare_dram_parameter("gwt", [GCH * 128, 96], BF16, isOutput=False)
    gb_d = nc.declare_dram_parameter("gb", [96], F32, isOutput=False)
    # fusion weight pre-packed as [o_blk, c_blk, 128c, 128o]
    fwt_d = nc.declare_dram_parameter("fwt", [NOB, NOB, 128, 128], BF16, isOutput=False)
    fb_d = nc.declare_dram_parameter("fb", [CO], F32, isOutput=False)
    out_d = nc.declare_dram_parameter("out", [CO, HW], F32, isOutput=True)

    cross_d = [nc.dram_tensor(f"cross{m}", [C, HW], BF16) for m in range(3)]
    s_d = {}
    if fast_qk:
        for i, (mq, mk) in enumerate(
            [(0, 1), (0, 2), (1, 0), (1, 2), (2, 0), (2, 1)]
        ):
            s_d[(mq, mk)] = nc.dram_tensor(f"s{mq}{mk}", [C, C], BF16)

    with tile.TileContext(nc) as tc:
        with (
            tc.tile_pool(name="pbig", bufs=1) as pbig,
            tc.tile_pool(name="pcr", bufs=1) as pcr,
            tc.tile_pool(name="pqk", bufs=3) as pqk,
            tc.tile_pool(name="pw", bufs=1) as pw,
            tc.tile_pool(name="patt", bufs=2) as patt,
            tc.tile_pool(name="pstream", bufs=4) as pstream,
            tc.tile_pool(name="pfw", bufs=16) as pfw,
            tc.tile_pool(name="pmisc", bufs=1) as pmisc,
            tc.tile_pool(name="pfin", bufs=3) as pfin,
            tc.tile_pool(name="pxt", bufs=2) as pxt,
            tc.tile_pool(name="psum", bufs=8, space="PSUM") as psum,
        ):
            # ---------- constants ----------
            ones_col = pmisc.tile([128, 1], BF16, tag="ones_col")
            nc.vector.memset(ones_col, 1.0)
            ones_col_f = pmisc.tile([128, 1], F32, tag="ones_col_f")
            nc.vector.memset(ones_col_f, 1.0)
            ones_k1_f = pmisc.tile([1, 128], F32, tag="ones_k1_f")
            nc.vector.memset(ones_k1_f, 1.0)
            ones_f32 = pmisc.tile([4, 512], F32, tag="ones_f32")
            nc.vector.memset(ones_f32, 1.0)

            # ---------- load x residents ----------
            xt = []  # xt[m][c] : [128, HW] bf16
            for m in range(3):
                tiles = []
                for c in range(NB):
                    t = pbig.tile([128, HW], BF16, tag=f"big{m * NB + c}")
                    nc.sync.dma_start(out=t, in_=x_d[m][c * 128:(c + 1) * 128, :])
                    tiles.append(t)
                xt.append(tiles)

            # optional row-bias tiles for QT/KT drains (bias varies along the
            # free dim in transposed layout -> need broadcast rows)
            def row_bias(dram_row, tag):
                bt = pmisc.tile([128, C], F32, tag=tag)
                src = bass.AP(
                    tensor=dram_row.tensor,
                    offset=dram_row.offset,
                    ap=[[0, 128]] + dram_row.ap,
                )
                nc.gpsimd.dma_start(out=bt, in_=src)
                return bt

            # ---------- S phase: shared pairwise channel grams ----------
            if fast_qk:
                for ma, mb in [(0, 1), (0, 2), (1, 2)]:
                    sacc = {
                        (ma, mb): [
                            psum.tile([128, 512], F32, tag="mm",
                                      name=f"sa{ma}{mb}_{a}")
                            for a in range(NB)
                        ],
                        (mb, ma): [
                            psum.tile([128, 512], F32, tag="mm",
                                      name=f"sa{mb}{ma}_{a}")
                            for a in range(NB)
                        ],
                    }
                    for nb in range(NBLK):
                        ta = pxt.tile([128, 512], BF16, tag="xta")
                        nc.sync.dma_start(
                            out=ta, in_=xT_d[ma][nb * 128:(nb + 1) * 128, :]
                        )
                        tb = pxt.tile([128, 512], BF16, tag="xtb")
                        nc.sync.dma_start(
                            out=tb, in_=xT_d[mb][nb * 128:(nb + 1) * 128, :]
                        )
                        for a in range(NB):
                            nc.tensor.matmul(
                                sacc[(ma, mb)][a],
                                lhsT=ta[:, a * 128:(a + 1) * 128], rhs=tb,
                                start=(nb == 0), stop=(nb == NBLK - 1),
                            )
                        for a in range(NB):
                            nc.tensor.matmul(
                                sacc[(mb, ma)][a],
                                lhsT=tb[:, a * 128:(a + 1) * 128], rhs=ta,
                                start=(nb == 0), stop=(nb == NBLK - 1),
                            )
                    for ki, key in enumerate(((ma, mb), (mb, ma))):
                        for a in range(NB):
                            dr = pxt.tile([128, 512], BF16, tag="sdrain",
                                          bufs=2)
                            if (ki + a) % 2 == 0:
                                nc.vector.tensor_copy(out=dr, in_=sacc[key][a])
                            else:
                                nc.scalar.copy(out=dr, in_=sacc[key][a])
                            nc.sync.dma_start(
                                out=s_d[key][a * 128:(a + 1) * 128, :], in_=dr
                            )

            # ---------- per-module pipeline ----------
            cross = {}  # modality -> list of 4 [128,HW] bf16 tiles
            for mi, (i, mq, mk) in enumerate(MODS[:nmods]):
                first_of_pair = (mi % 2) == 0
                fused_init = first_of_pair and not crossb_nz
                if first_of_pair:
                    # cross accumulator for this query modality; with zero
                    # proj bias the x residual is fused into this module's
                    # phase-C adds instead of an upfront copy
                    crt = []
                    for c in range(NB):
                        t = pcr.tile([128, HW], BF16, tag=f"cr{c}")
                        if crossb_nz:
                            cb = pmisc.tile([128, 1], F32, tag="crossb_t")
                            nc.sync.dma_start(
                                out=cb,
                                in_=crossb_d[mq, c * 128:(c + 1) * 128].rearrange(
                                    "a -> a 1"
                                ),
                            )
                            nc.vector.tensor_scalar_add(t, xt[mq][c], cb)
                        crt.append(t)
                    cross[mq] = crt

                # weights for this module
                wq = []
                wk = []
                wv = []
                wo = []
                for c in range(NB):
                    for lst, src, nm, nbuf in (
                        (wq, wqt_d, "wq", 1), (wk, wkt_d, "wk", 1),
                        (wv, wvt_d, "wv", 1), (wo, wot_d, "wo", 1),
                    ):
                        t = pw.tile([128, C], BF16, tag=f"{nm}{c}", bufs=nbuf)
                        nc.sync.dma_start(
                            out=t, in_=src[i, c * 128:(c + 1) * 128, :]
                        )
                        lst.append(t)

                if fast_qk:
                    # ---- phase A': G^T = Wk S' Wq^T via shared S ----
                    s_in = []
                    for a in range(NB):
                        t = pxt.tile([128, 512], BF16, tag=f"sin{a}", bufs=1)
                        nc.sync.dma_start(
                            out=t, in_=s_d[(mq, mk)][a * 128:(a + 1) * 128, :]
                        )
                        s_in.append(t)
                    # step 1: M[b, dq] = sum_a S_qk[a, b] Wq[dq, a]
                    m_sb = []
                    for b in range(NB):
                        psm = psum.tile([128, 512], F32, tag="mm")
                        for a in range(NB):
                            nc.tensor.matmul(
                                psm, lhsT=s_in[a][:, b * 128:(b + 1) * 128],
                                rhs=wq[a],
                                start=(a == 0), stop=(a == NB - 1),
                            )
                        mt = pxt.tile([128, 512], BF16, tag=f"m{b}", bufs=1)
                        if b % 2 == 0:
                            nc.vector.tensor_copy(out=mt, in_=psm)
                        else:
                            nc.scalar.copy(out=mt, in_=psm)
                        m_sb.append(mt)
                    # step 2: G^T[dk, dq] = sum_b Wk[dk, b] M[b, dq],
                    # head-diagonal blocks only, in pair-tile gram layout
                    gram = [psum.tile([128, 512], F32, tag="mm",
                                      name=f"gram{g}") for g in range(NB)]
                    for h in range(HEADS):
                        mc = h // 2
                        r0 = (h % 2) * HD
                        dst = gram[mc][r0:r0 + HD, h * HD:(h + 1) * HD]
                        for b in range(NB):
                            nc.tensor.matmul(
                                dst, lhsT=wk[b][:, h * HD:(h + 1) * HD],
                                rhs=m_sb[b][:, h * HD:(h + 1) * HD],
                                start=(b == 0), stop=(b == NB - 1),
                            )
                else:
                    bq_t = row_bias(bq_d[i, :], "bq_row") if bq_nz else None
                    bk_t = row_bias(bk_d[i, :], "bk_row") if bk_nz else None

                    # ---- phase A: QT/KT convs + Gram^T accumulation ----
                    gram = [psum.tile([128, 512], F32, tag="mm",
                                      name=f"gram{g}") for g in range(NB)]
                    for nb in range(NBLK):
                        ns = slice(nb * 128, (nb + 1) * 128)
                        psq = psum.tile([128, 512], F32, tag="mm")
                        for c in range(NB):
                            nc.tensor.matmul(
                                psq, lhsT=xt[mq][c][:, ns], rhs=wq[c],
                                start=(c == 0), stop=(c == NB - 1),
                            )
                        qt_sb = pqk.tile([128, 512], BF16, tag="qt")
                        if bq_nz:
                            nc.vector.tensor_add(qt_sb, psq, bq_t)
                        else:
                            nc.scalar.copy(out=qt_sb, in_=psq)

                        psk = psum.tile([128, 512], F32, tag="mm")
                        for c in range(NB):
                            nc.tensor.matmul(
                                psk, lhsT=xt[mk][c][:, ns], rhs=wk[c],
                                start=(c == 0), stop=(c == NB - 1),
                            )
                        kt_sb = pqk.tile([128, 512], BF16, tag="kt")
                        if bk_nz:
                            nc.vector.tensor_add(kt_sb, psk, bk_t)
                        else:
                            nc.vector.tensor_copy(out=kt_sb, in_=psk)

                        # Gram^T[a,b] += sum_n KT[n,a] QT[n,b]
                        for mc in range(NB):
                            nc.tensor.matmul(
                                gram[mc],
                                lhsT=kt_sb[:, mc * 128:(mc + 1) * 128],
                                rhs=qt_sb,
                                start=(nb == 0), stop=(nb == NBLK - 1),
                            )

                # ---- phase B: softmax -> attn^T block-diagonal pair tiles ----
                pair = []
                for p in range(NB):
                    pt = patt.tile([128, 128], BF16, tag=f"pd{p}")
                    nc.vector.memset(pt, 0.0)
                    pair.append(pt)
                for h in range(HEADS):
                    mc = h // 2
                    r0 = (h % 2) * HD
                    blk = gram[mc][r0:r0 + HD, h * HD:(h + 1) * HD]
                    e_sb = pstream.tile([HD, HD], F32, tag="esb")
                    nc.scalar.activation(
                        out=e_sb, in_=blk, func=AF.Exp, scale=SCALE
                    )
                    pss = psum.tile([1, HD], F32, tag="mm")
                    nc.tensor.matmul(pss, lhsT=ones_col_f[:HD, :], rhs=e_sb,
                                     start=True, stop=True)
                    r_sb = pstream.tile([1, HD], F32, tag="rsb")
                    nc.vector.reciprocal(out=r_sb, in_=pss)
                    psr = psum.tile([HD, HD], F32, tag="mm")
                    nc.tensor.matmul(psr, lhsT=ones_k1_f[:, :HD], rhs=r_sb,
                                     start=True, stop=True)
                    nc.vector.tensor_mul(
                        pair[mc][r0:r0 + HD, r0:r0 + HD], e_sb, psr
                    )

                bv_t = None
                if bv_nz:
                    bv_t = pmisc.tile([128, NB], F32, tag="bv_t")
                    nc.sync.dma_start(
                        out=bv_t,
                        in_=bv_d[i, :].rearrange("(c p) -> p c", p=128),
                    )

                # ---- phase C: V conv + attn@V + out-proj, per n-tile ----
                for nt in range(NT):
                    ts_ = slice(nt * 512, (nt + 1) * 512)
                    ao = []
                    for p in range(NB):
                        psv = psum.tile([128, 512], F32, tag="mm")
                        for c in range(NB):
                            nc.tensor.matmul(
                                psv,
                                lhsT=wv[c][:, p * 128:(p + 1) * 128],
                                rhs=xt[mk][c][:, ts_],
                                start=(c == 0), stop=(c == NB - 1),
                            )
                        v_sb = pstream.tile([128, 512], BF16, tag="vsb", bufs=3)
                        if bv_nz:
                            nc.vector.tensor_scalar_add(
                                v_sb, psv, bv_t[:, p:p + 1]
                            )
                        else:
                            nc.scalar.copy(out=v_sb, in_=psv)
                        psa = psum.tile([128, 512], F32, tag="mm")
                        nc.tensor.matmul(psa, lhsT=pair[p], rhs=v_sb,
                                         start=True, stop=True)
                        ao_sb = pstream.tile([128, 512], BF16, tag=f"ao{p}", bufs=2)
                        nc.vector.tensor_copy(out=ao_sb, in_=psa)
                        ao.append(ao_sb)
                    for o in range(NB):
                        psp = psum.tile([128, 512], F32, tag="mm")
                        for p in range(NB):
                            nc.tensor.matmul(
                                psp,
                                lhsT=wo[p][:, o * 128:(o + 1) * 128],
                                rhs=ao[p],
                                start=(p == 0), stop=(p == NB - 1),
                            )
                        nc.vector.tensor_add(
                            cross[mq][o][:, ts_], psp,
                            xt[mq][o][:, ts_] if fused_init
                            else cross[mq][o][:, ts_],
                        )

                if mi % 2 == 1 and mq < 2:
                    # cross accumulator complete -> spill to DRAM scratch
                    # (mq=2 stays live in pcr; refined[2] aliases it)
                    for c in range(NB):
                        nc.sync.dma_start(
                            out=cross_d[mq][c * 128:(c + 1) * 128, :],
                            in_=cross[mq][c],
                        )

            if not do_final:
                for c in range(NB):
                    nc.gpsimd.dma_start(
                        out=out_d[c * 128:(c + 1) * 128, :],
                        in_=cross[0][c])
            else:
                if fin_stage >= 1:
                    # ---------- quality metrics (from bf16 residents) ----------
                    # per-modality scalars: depth zeros count, rgb var, lidar zeros
                    scr = pmisc.tile([128, 512], BF16, tag="q_scr")
                    red = pmisc.tile([128, 3 * NB], F32, tag="q_red")  # per-cblk sums
                    red2 = pmisc.tile([128, NB], F32, tag="q_red2")    # rgb sum x
                    zacc = pmisc.tile([128, NT], F32, tag="q_zacc")
                    sq_part = pmisc.tile([128, 512], BF16, tag="q_sqp")
                    if q_stage >= 1:
                        for slot, xm in ((0, 1), (2, 2)):  # depth zeros, lidar zeros
                            for c in range(NB):
                                for j in range(NT):
                                    ts_ = slice(j * 512, (j + 1) * 512)
                                    nc.vector.tensor_scalar(
                                        scr, xt[xm][c][:, ts_], 0.0, None,
                                        op0=ALU.is_equal,
                                    )
                                    nc.vector.reduce_sum(
                                        out=zacc[:, j:j + 1], in_=scr,
                                        axis=mybir.AxisListType.X,
                                    )
                                nc.vector.reduce_sum(
                                    out=red[:, slot * NB + c:slot * NB + c + 1],
                                    in_=zacc, axis=mybir.AxisListType.X,
                                )
                    if q_stage >= 2:
                        for c in range(NB):  # rgb sum-of-squares and sum
                            acc = pmisc.tile([128, NT], F32, tag="q_chain")
                            for j in range(NT):
                                ts_ = slice(j * 512, (j + 1) * 512)
                                nc.vector.tensor_mul(
                                    sq_part, xt[0][c][:, ts_], xt[0][c][:, ts_]
                                )
                                nc.vector.reduce_sum(
                                    out=acc[:, j:j + 1], in_=sq_part,
                                    axis=mybir.AxisListType.X,
                                )
                            nc.vector.reduce_sum(
                                out=red[:, NB + c:NB + c + 1], in_=acc,
                                axis=mybir.AxisListType.X,
                            )
                            nc.vector.reduce_sum(
                                out=red2[:, c:c + 1], in_=xt[0][c],
                                axis=mybir.AxisListType.X,
                            )
                    if q_stage >= 3:
                        # column-sum the [128, *] partial sums via ones matmul
                        psq1 = psum.tile([1, 3 * NB], F32, tag="mm")
                        nc.tensor.matmul(psq1, lhsT=ones_col_f, rhs=red,
                                         start=True, stop=True)
                        psq2 = psum.tile([1, NB], F32, tag="mm")
                        nc.tensor.matmul(psq2, lhsT=ones_col_f, rhs=red2,
                                         start=True, stop=True)
                        qsc = pmisc.tile([1, 8], F32, tag="q_sc")
                        # [0]=dep_zeros, [1]=rgb_sumsq, [2]=lid_zeros, [3]=rgb_sum
                        nc.vector.reduce_sum(out=qsc[:, 0:1], in_=psq1[:, 0:NB],
                                             axis=mybir.AxisListType.X)
                        nc.vector.reduce_sum(out=qsc[:, 1:2], in_=psq1[:, NB:2 * NB],
                                             axis=mybir.AxisListType.X)
                        nc.vector.reduce_sum(out=qsc[:, 2:3], in_=psq1[:, 2 * NB:3 * NB],
                                             axis=mybir.AxisListType.X)
                        nc.vector.reduce_sum(out=qsc[:, 3:4], in_=psq2,
                                             axis=mybir.AxisListType.X)
                        # var = (S2 - S1^2/N) / (N-1)
                        nc.vector.tensor_mul(qsc[:, 4:5], qsc[:, 3:4], qsc[:, 3:4])
                        nc.vector.tensor_scalar(
                            qsc[:, 4:5], qsc[:, 4:5], -1.0 / NTOT, None, op0=ALU.mult
                        )
                        nc.vector.tensor_add(qsc[:, 4:5], qsc[:, 4:5], qsc[:, 1:2])
                        nc.vector.tensor_scalar(
                            qsc[:, 4:5], qsc[:, 4:5], 1.0 / (NTOT - 1), None, op0=ALU.mult
                        )
                        # quality chunk [128,512]: row0 dep_sparsity, row1 rgb_var,
                        # row2 lid_sparsity, rows 3-127 zero; constant along n.
                    if q_stage >= 4:
                        qual = pmisc.tile([128, 512], BF16, tag="qual")
                        nc.vector.memset(qual, 0.0)
                        nc.vector.tensor_scalar(
                            qual[0:1, :], ones_f32[0:1, :], qsc[:, 0:1], 1.0 / NTOT,
                            op0=ALU.mult, op1=ALU.mult,
                        )
                        nc.vector.tensor_scalar(
                            qual[32:33, :], ones_f32[0:1, :], qsc[:, 4:5], None,
                            op0=ALU.mult,
                        )
                        nc.vector.tensor_scalar(
                            qual[64:65, :], ones_f32[0:1, :], qsc[:, 2:3], 1.0 / NTOT,
                            op0=ALU.mult, op1=ALU.mult,
                        )

                if fin_stage >= 2:
                    # ---------- final phase: SE, l2norm, gate, fusion ----------
                    refined = []  # [m][c] -> [128, HW] bf16 (reuses x resident slots)
                    for m in range(3):
                        if m == 2 and 2 in cross:
                            refined.append(cross[2])
                            continue
                        tiles = []
                        for c in range(NB):
                            t = pbig.tile([128, HW], BF16, tag=f"big{m * NB + c}")
                            nc.sync.dma_start(
                                out=t, in_=cross_d[m][c * 128:(c + 1) * 128, :]
                            )
                            tiles.append(t)
                        refined.append(tiles)

                    for m in range(3):
                        # SE block
                        pooled = pmisc.tile([128, NB], F32, tag="se_pool")
                        for c in range(NB):
                            nc.vector.reduce_sum(
                                out=pooled[:, c:c + 1], in_=refined[m][c],
                                axis=mybir.AxisListType.X,
                            )
                        pooled_bf = pmisc.tile([128, NB], BF16, tag="se_pool_bf")
                        nc.vector.tensor_copy(out=pooled_bf, in_=pooled)
                        w1 = pmisc.tile([128, NB, 32], BF16, tag="se_w1")
                        nc.sync.dma_start(
                            out=w1,
                            in_=sew1t_d[m].rearrange("(c p) o -> p c o", p=128),
                        )
                        ps_y = psum.tile([32, 1], F32, tag="mm")
                        for c in range(NB):
                            nc.tensor.matmul(
                                ps_y, lhsT=w1[:, c, :],
                                rhs=pooled_bf[:, c:c + 1],
                                start=(c == 0), stop=(c == NB - 1),
                            )
                        y_sb = pmisc.tile([32, 1], BF16, tag="se_y")
                        if seb1_nz:
                            b1t = pmisc.tile([32, 1], F32, tag="se_b1")
                            nc.sync.dma_start(
                                out=b1t, in_=seb1_d[m, :].rearrange("a -> a 1")
                            )
                            nc.scalar.activation(out=y_sb, in_=ps_y, func=AF.Relu,
                                                 scale=1.0 / HW, bias=b1t)
                        else:
                            nc.scalar.activation(out=y_sb, in_=ps_y, func=AF.Relu,
                                                 scale=1.0 / HW)
                        w2 = pmisc.tile([32, C], BF16, tag="se_w2")
                        nc.sync.dma_start(out=w2, in_=sew2t_d[m])
                        scale_sb = pmisc.tile([128, NB], F32, tag="se_scale")
                        for c in range(NB):
                            ps_s = psum.tile([128, 1], F32, tag="mm")
                            nc.tensor.matmul(
                                ps_s, lhsT=w2[:, c * 128:(c + 1) * 128], rhs=y_sb,
                                start=True, stop=True,
                            )
                            if seb2_nz:
                                b2t = pmisc.tile([128, NB], F32, tag="se_b2")
                                nc.sync.dma_start(
                                    out=b2t[:, c:c + 1],
                                    in_=seb2_d[m, c * 128:(c + 1) * 128].rearrange(
                                        "a -> a 1"
                                    ),
                                )
                                nc.scalar.activation(out=scale_sb[:, c:c + 1],
                                                     in_=ps_s, func=AF.Sigmoid,
                                                     scale=1.0, bias=b2t[:, c:c + 1])
                            else:
                                nc.scalar.activation(out=scale_sb[:, c:c + 1],
                                                     in_=ps_s, func=AF.Sigmoid)
                        # refined_pre = cross * se_scale (in place, bf16)
                        for c in range(NB):
                            nc.vector.tensor_scalar_mul(
                                refined[m][c], refined[m][c], scale_sb[:, c:c + 1]
                            )
                        # l2 norm over channels, processed per n-tile
                        for nt in range(NT):
                            ts_ = slice(nt * 512, (nt + 1) * 512)
                            ps_ss = psum.tile([1, 512], F32, tag="mm")
                            for c in range(NB):
                                sq = pqk.tile([128, 512], BF16, tag="qt")
                                nc.vector.tensor_mul(
                                    sq, refined[m][c][:, ts_], refined[m][c][:, ts_]
                                )
                                nc.tensor.matmul(ps_ss, lhsT=ones_col, rhs=sq,
                                                 start=(c == 0), stop=(c == NB - 1))
                            ssn = pstream.tile([1, 512], F32, tag="ssn", bufs=2)
                            nc.scalar.activation(out=ssn, in_=ps_ss, func=AF.Sqrt)
                            nc.vector.tensor_scalar_add(ssn, ssn, EPS)
                            nc.vector.reciprocal(out=ssn, in_=ssn)
                            ps_rep = psum.tile([128, 512], F32, tag="mm")
                            nc.tensor.matmul(ps_rep, lhsT=ones_k1_f, rhs=ssn,
                                             start=True, stop=True)
                            for c in range(NB):
                                nc.vector.tensor_mul(
                                    refined[m][c][:, ts_], refined[m][c][:, ts_],
                                    ps_rep,
                                )

                if fin_stage >= 3:
                    # gate conv -> alpha [3, HW]
                    gw = []
                    for ch in range(GCH):
                        t = pmisc.tile([128, 96], BF16, tag=f"gw{ch}")
                        nc.sync.dma_start(out=t, in_=gwt_d[ch * 128:(ch + 1) * 128, :])
                        gw.append(t)
                    gb_t = None
                    if gb_nz:
                        gb_t = pmisc.tile([96, 1], F32, tag="gb_t")
                        nc.sync.dma_start(out=gb_t, in_=gb_d.rearrange("a -> a 1"))
                    for nt in range(NT):
                        ts_ = slice(nt * 512, (nt + 1) * 512)
                        ps_g = psum.tile([96, 512], F32, tag="mm")
                        for ch in range(GCH):
                            if ch < 12:
                                rhs = refined[ch // NB][ch % NB][:, ts_]
                            else:
                                rhs = qual
                            nc.tensor.matmul(ps_g, lhsT=gw[ch], rhs=rhs,
                                             start=(ch == 0), stop=(ch == GCH - 1))
                        for m in range(3):
                            a_nt = pstream.tile([1, 512], F32, tag="a",
                                                name=f"a{m}", bufs=3)
                            if gb_nz:
                                nc.scalar.activation(out=a_nt, in_=ps_g[32 * m:32 * m + 1, :],
                                                     func=AF.Sigmoid, scale=1.0,
                                                     bias=gb_t[32 * m:32 * m + 1, :])
                            else:
                                nc.scalar.activation(out=a_nt, in_=ps_g[32 * m:32 * m + 1, :],
                                                     func=AF.Sigmoid)
                            ps_ar = psum.tile([128, 512], F32, tag="mm")
                            nc.tensor.matmul(ps_ar, lhsT=ones_k1_f, rhs=a_nt,
                                             start=True, stop=True)
                            for c in range(NB):
                                nc.vector.tensor_mul(
                                    refined[m][c][:, ts_], refined[m][c][:, ts_],
                                    ps_ar,
                                )

                if fin_stage >= 4:
                    # fusion conv: out = fusion_W @ weighted (+ fb)
                    fb_t = None
                    if fb_nz:
                        fb_t = pmisc.tile([128, NOB], F32, tag="fb_t")
                        nc.sync.dma_start(
                            out=fb_t, in_=fb_d.rearrange("(o p) -> p o", p=128)
                        )
                    for o in range(NOB):
                        fw = []
                        for cb in range(NOB):
                            t = pfw.tile([128, 128], BF16, tag="fw")
                            nc.sync.dma_start(out=t, in_=fwt_d[o, cb])
                            fw.append(t)
                        for nt in range(NT):
                            ts_ = slice(nt * 512, (nt + 1) * 512)
                            ps_f = psum.tile([128, 512], F32, tag="mm")
                            for cb in range(NOB):
                                nc.tensor.matmul(
                                    ps_f, lhsT=fw[cb],
                                    rhs=refined[cb // NB][cb % NB][:, ts_],
                                    start=(cb == 0), stop=(cb == NOB - 1),
                                )
                            o_sb = pfin.tile([128, 512], F32, tag="osb")
                            if fb_nz:
                                nc.vector.tensor_scalar_add(o_sb, ps_f,
                                                            fb_t[:, o:o + 1])
                            elif (o + nt) % 2 == 0:
                                nc.vector.tensor_copy(out=o_sb, in_=ps_f)
                            else:
                                nc.scalar.copy(out=o_sb, in_=ps_f)
                            nc.sync.dma_start(
                                out=out_d[o * 128:(o + 1) * 128, ts_], in_=o_sb
                            )
    nc.finalize()
    return nc


def _get_graph(flags):
    if flags not in _CACHE:
        _CACHE[flags] = _build(flags)
    return _CACHE[flags]


def kernel(**inputs):
    bf16 = ml_dtypes.bfloat16
    f32 = np.float32

    rgb = np.asarray(inputs["rgb_features"], f32)
    dep = np.asarray(inputs["depth_features"], f32)
    lid = np.asarray(inputs["lidar_features"], f32)
    Wq = np.asarray(inputs["attn_Wq"], f32)
    bq = np.asarray(inputs["attn_bq"], f32)
    Wk = np.asarray(inputs["attn_Wk"], f32)
    bk = np.asarray(inputs["attn_bk"], f32)
    Wv = np.asarray(inputs["attn_Wv"], f32)
    bv = np.asarray(inputs["attn_bv"], f32)
    Wo = np.asarray(inputs["attn_Wo"], f32)
    bo = np.asarray(inputs["attn_bo"], f32)
    seW1 = np.asarray(inputs["se_W1"], f32)
    seb1 = np.asarray(inputs["se_b1"], f32)
    seW2 = np.asarray(inputs["se_W2"], f32)
    seb2 = np.asarray(inputs["se_b2"], f32)
    gW = np.asarray(inputs["gate_W"], f32)
    gb = np.asarray(inputs["gate_b"], f32)
    fW = np.asarray(inputs["fusion_W"], f32)
    fb = np.asarray(inputs["fusion_b"], f32)

    gb96 = np.zeros(96, f32)
    gb96[[0, 32, 64]] = gb
    crossb = np.stack([bo[0] + bo[1], bo[2] + bo[3], bo[4] + bo[5]])
    flags = (
        bool(bq.any()), bool(bk.any()), bool(bv.any()), bool(crossb.any()),
        bool(seb1.any()), bool(seb2.any()), bool(gb.any()), bool(fb.any()),
    )
    nc = _get_graph(flags)

    wqt = np.ascontiguousarray(Wq.transpose(0, 2, 1)).astype(bf16)
    wkt = np.ascontiguousarray(Wk.transpose(0, 2, 1)).astype(bf16)
    wvt = np.ascontiguousarray(Wv.transpose(0, 2, 1)).astype(bf16)
    wot = np.ascontiguousarray(Wo.transpose(0, 2, 1)).astype(bf16)
    sew1t = np.ascontiguousarray(seW1.transpose(0, 2, 1)).astype(bf16)
    sew2t = np.ascontiguousarray(seW2.transpose(0, 2, 1)).astype(bf16)
    gwt = np.zeros((GCH * 128, 96), f32)
    for m3 in range(3):
        gwt[:3 * C, 32 * m3] = gW.T[:3 * C, m3]
        for q3 in range(3):
            gwt[12 * 128 + 32 * q3, 32 * m3] = gW.T[3 * C + q3, m3]
    gwt = gwt.astype(bf16)
    fwt = np.ascontiguousarray(
        fW.T.reshape(NOB, 128, NOB, 128).transpose(2, 0, 1, 3)
    ).astype(bf16)

    shared = {
        "wqt": wqt, "wkt": wkt, "wvt": wvt, "wot": wot,
        "bq": bq, "bk": bk, "bv": bv, "crossb": crossb,
        "sew1t": sew1t, "sew2t": sew2t, "seb1": seb1, "seb2": seb2,
        "gwt": gwt, "gb": gb96, "fwt": fwt, "fb": fb,
    }
    in_maps = []
    for b in range(B):
        m = dict(shared)
        for key, arr in (("x0", rgb), ("x1", dep), ("x2", lid)):
            xb = arr[b].reshape(C, HW).astype(bf16)
            m[key] = xb
            m["xt" + key[1]] = np.ascontiguousarray(xb.T)
        in_maps.append(m)

    global _last_in_maps
    _last_in_maps = in_maps
    res = run_bass_kernel_spmd(nc, in_maps, core_ids=list(range(B)))
    out = np.stack([res.results[b]["out"] for b in range(B)])
    return out.reshape(B, CO, H, W).astype(np.float32)

